# revision 33
# baseline (speedup 1.0000x reference)
"""HGRNBitMLP (BitNet-style SwiGLU MLP) on 8 TRN2 NeuronCores.

Data-parallel over the 4096 tokens (512/core). Weight ternarization is
COLUMN-sharded: core c ternarizes gate cols [c*1024,(c+1)*1024) plus the
matching v cols, so the per-chunk AllGathers fire as soon as the local
tern finishes (~80us) instead of waiting for a row-wise union. Ternary
weights ride fp8e4 (exact for {-1,0,+1}); the global mean(|w|) comes
from two tiny AllGathers (gate stats first so the gate tern never waits
on the down-proj read). Activations are quantized to the int8 grid
(exact in bf16), so every matmul is an exact-integer fp8xbf16 matmul
with f32 PSUM accumulation; per-token scales are applied outside.

Engine-queue discipline (strict FIFO per engine):
 - sync(SP) ring:   w-gate stat rows i<8 (held in SBUF for tern) ->
                    gate tern re-reads rows i>=8 chunks 0-3 ->
                    mm1 stationary tiles -> mm2 h reads + y evens.
 - scalar(ACT) ring: x -> w-gate stat rows i>=8 -> w-down (held) ->
                    gate tern re-reads chunks 4-5 -> mm1 h spills ->
                    mm2 td loads + y odds.
 - Pool queue: collective triggers (in chunk order), tern chunks 2-5,
                    down stats ... down tern, fp8 tern writes (SWDGE),
                    mm2 magic-rounds, broadcasts.
 - DVE: P0x helpers -> tern chunks 0-1 -> down-stat reduces -> mm1
                    SwiGLU drains -> P4 -> mm2 quant muls + yt scales.

Layouts: x is loaded [tok, h], quantized, PE-transposed to xqT [h, tok].
mm1 produces y^T tiles [o, tok]; SwiGLU keeps h as [I, tok] (f32,
spilled to DRAM) so mm2's operand q2T [I, tok] needs no transpose.
mm2 is j-outer over I with quantization fused into the first H-half
(q2T cached in SBUF bf16 for the second half), out [tok, H] in halves.
"""
import sys

try:
    import concourse  # noqa: F401
except ImportError:
    sys.path.insert(0, "/opt/trn_rl_repo")

import numpy as np

import concourse.tile as tile
from concourse import bacc, mybir
from concourse.bass_utils import run_bass_kernel_spmd
from concourse.masks import make_identity

F32, BF16, F8 = mybir.dt.float32, mybir.dt.bfloat16, mybir.dt.float8e4
Alu = mybir.AluOpType
Act = mybir.ActivationFunctionType
X = mybir.AxisListType.X

NC_N = 8
B, S, H, I = 2, 2048, 2048, 8192
O2 = 2 * I
TOK = B * S
TPC = TOK // NC_N   # 512 tokens/core
TT = TPC // 128     # 4 token tiles
HK = H // 128       # 16 h tiles
IK = I // 128       # 64 I tiles
GSH = H             # rows of the (column-sharded) gate slice
GCW = I // NC_N     # 1024 gate cols (and v cols) per core
DSH = I // NC_N     # 1024 rows of w_down^T per core
EPS = 1e-5
C_MAGIC = 12582912.0  # 1.5*2^23; (x+C)-C rounds f32 to nearest-even int

# gate AG chunks: gate-col widths (same width of v cols rides along)
CW = [128, 128, 128, 128, 256, 256]
GO = [0, 128, 256, 384, 512, 768]
NCH = len(CW)


def build(nc):
    x_ap = nc.dram_tensor("x", [TPC, H], F32, kind="ExternalInput").ap()
    wg_ap = nc.dram_tensor("wgt", [GSH, 2 * GCW], F32, kind="ExternalInput").ap()
    wd_ap = nc.dram_tensor("wdt", [DSH, H], F32, kind="ExternalInput").ap()
    gg_ap = nc.dram_tensor("gg", [1, H], F32, kind="ExternalInput").ap()
    gd_ap = nc.dram_tensor("gdc", [128, IK], F32, kind="ExternalInput").ap()
    y_ap = nc.dram_tensor("y", [TPC, H], F32, kind="ExternalOutput").ap()
    rg = [list(range(NC_N))]

    with tile.TileContext(nc) as tc:
        with tc.tile_pool(name="dram", bufs=1, space="DRAM") as dram, \
             tc.tile_pool(name="perm", bufs=1) as cp, \
             tc.tile_pool(name="colp", bufs=1) as colp:

            # ---- inits (before any collective hits the Pool queue) --
            ones = cp.tile([128, 1], F32)
            nc.gpsimd.memset(ones[:], 1.0)
            epsb = cp.tile([128, 1], F32)
            nc.gpsimd.memset(epsb[:], EPS)
            ident_b = cp.tile([128, 128], BF16)
            make_identity(nc, ident_b[:])
            ident_f = cp.tile([128, 128], F32)
            make_identity(nc, ident_f[:])
            gdc_sb = cp.tile([128, IK], F32)
            nc.gpsimd.dma_start(gdc_sb[:], gd_ap[:])
            gg_sb = cp.tile([1, H], F32)
            nc.gpsimd.dma_start(gg_sb[:], gg_ap[:])
            g_bc = cp.tile([128, H], F32)
            nc.gpsimd.partition_broadcast(g_bc[:], gg_sb[:])

            # warmup collective: the first collective after the init
            # barrier pays ~20us extra; burn that on a dummy.
            warm_sb = cp.tile([1, 1], F32)
            nc.vector.memset(warm_sb[:], 0.0)
            warm_in = dram.tile([1, 1], F32, name="warm_in")
            warm_out = dram.tile([8, 1], F32, addr_space="Shared",
                                 name="warm_out")
            nc.sync.dma_start(warm_in[:], warm_sb[:])
            nc.gpsimd.collective_compute("AllGather", Alu.bypass,
                                         replica_groups=rg,
                                         ins=[warm_in[:]], outs=[warm_out[:]])

            # ---- P0x: x rmsnorm + int8-grid quant + transpose.
            # Emitted FIRST on ACT/DVE so the x chain races ahead of the
            # stats reduces; x rides the scalar ring before the w tiles.
            xq_p = tc.tile_pool(name="xqp", bufs=1)
            xq_pool = xq_p.__enter__()
            xqT = xq_pool.tile([128, HK * TPC], BF16)
            amax1 = colp.tile([128, TT], F32)
            # statp opens BEFORE xwork so its tiles get their own SBUF
            # region: reusing xwork's space would gate every stats DMA
            # on P0x completion (WAR) and stall the whole startup.
            stp_ctx = tc.tile_pool(name="statp", bufs=6)
            stp = stp_ctx.__enter__()
            xw_ctx = tc.tile_pool(name="xwork", bufs=2)
            xw = xw_ctx.__enter__()
            psX_ctx = tc.tile_pool(name="psX", bufs=2, space="PSUM")
            psX = psX_ctx.__enter__()
            for t in range(TT):
                xt = xw.tile([128, H], F32, tag="xt", name=f"xt{t}")
                nc.scalar.dma_start(xt[:], x_ap[t * 128:(t + 1) * 128, :])
                ssq = colp.tile([128, 1], F32, name=f"ssq{t}")
                scr = xw.tile([128, H], F32, tag="scrA", name=f"sq{t}")
                nc.scalar.activation(scr[:], xt[:], Act.Square,
                                     accum_out=ssq[:])
                sd = colp.tile([128, 1], F32, name=f"sd{t}")
                nc.scalar.activation(sd[:], ssq[:], Act.Sqrt, bias=epsb[:],
                                     scale=1.0 / H)
                rstd = colp.tile([128, 1], F32, name=f"rstd{t}")
                nc.vector.reciprocal(rstd[:], sd[:])
                # fold rstd into the final quant scale: xn' = x*g, then
                # q = round(xn' * (rstd*127/clip(amax|xn'|*rstd, eps)))
                xn = xw.tile([128, H], F32, tag="xn", name=f"xn{t}")
                nc.vector.tensor_tensor(xn[:], xt[:], g_bc[:], Alu.mult)
                am = amax1[:, t:t + 1]
                nc.vector.tensor_reduce(am, xn[:], axis=X, op=Alu.max,
                                        apply_absolute_value=True)
                # amax1 holds amax|xnorm| = amax|x*g| * rstd, clipped
                nc.vector.tensor_scalar(am, am, rstd[:], EPS,
                                        Alu.mult, Alu.max)
                rc = colp.tile([128, 1], F32, name=f"rc{t}")
                nc.vector.reciprocal(rc[:], am)
                s1 = colp.tile([128, 1], F32, name=f"s1{t}")
                nc.vector.tensor_scalar(s1[:], rc[:], rstd[:], 127.0,
                                        Alu.mult, Alu.mult)
                # int8-grid round via the magic-constant trick on ACT
                q1 = xw.tile([128, H], F32, tag="scrA", name=f"q1{t}")
                nc.scalar.activation(q1[:], xn[:], Act.Copy,
                                     scale=s1[:], bias=C_MAGIC)
                q = xw.tile([128, H], BF16, tag="q", name=f"q{t}")
                nc.scalar.activation(q[:], q1[:], Act.Copy, bias=-C_MAGIC)
                for i in range(HK):
                    tps = psX.tile([128, 128], BF16, tag="tps",
                                   name=f"tps{t}_{i}")
                    nc.tensor.transpose(tps[:], q[:, i * 128:(i + 1) * 128],
                                        ident_b[:])
                    nc.scalar.copy(xqT[:, i * TPC + t * 128:
                                       i * TPC + (t + 1) * 128], tps[:])
            psX_ctx.__exit__(None, None, None)
            xw_ctx.__exit__(None, None, None)

            # ---- P0w: sharded abs-sums. Gate first (its own tiny AG)
            # so the gate tern never waits on the down-proj read. ----
            stat_g = colp.tile([1, 1], F32)
            stat_d = colp.tile([1, 1], F32)
            psS_ctx = tc.tile_pool(name="psStat", bufs=1, space="PSUM")
            psS = psS_ctx.__enter__()

            parts_g = colp.tile([128, HK], F32)
            for i in range(HK):
                wch = stp.tile([128, 2 * GCW], F32, tag="wst", name=f"ws{i}")
                eng = nc.sync if i % 2 == 0 else nc.scalar
                eng.dma_start(wch[:], wg_ap[i * 128:(i + 1) * 128, :])
                nc.vector.tensor_reduce(parts_g[:, i:i + 1], wch[:], axis=X,
                                        op=Alu.add, apply_absolute_value=True)
            sums_g = colp.tile([128, 1], F32)
            nc.vector.tensor_reduce(sums_g[:], parts_g[:], axis=X, op=Alu.add)
            psg = psS.tile([1, 1], F32, tag="psg")
            nc.tensor.matmul(psg[:], sums_g[:], ones[:], start=True, stop=True)
            nc.scalar.copy(stat_g[:], psg[:])
            sg_in = dram.tile([1, 1], F32, name="sgin")
            sg_out = dram.tile([8, 1], F32, addr_space="Shared", name="sgout")
            nc.sync.dma_start(sg_in[:], stat_g[:])
            nc.gpsimd.collective_compute("AllGather", Alu.bypass,
                                         replica_groups=rg,
                                         ins=[sg_in[:]], outs=[sg_out[:]])
            resg = colp.tile([8, 1], F32)
            nc.sync.dma_start(resg[:], sg_out[:])
            psg2 = psS.tile([1, 1], F32, tag="psg2")
            nc.tensor.matmul(psg2[:], ones[0:8, 0:1], resg[:],
                             start=True, stop=True)
            statg_res = colp.tile([1, 1], F32)
            nc.scalar.copy(statg_res[:], psg2[:])

            # ---- gate thresholds ----
            def bcast_scaled(src, scale, name):
                t1 = colp.tile([1, 1], F32, name=f"{name}_s")
                nc.vector.tensor_scalar_mul(t1[:], src, scale)
                t2 = colp.tile([128, 1], F32, name=f"{name}_b")
                nc.gpsimd.partition_broadcast(t2[:], t1[:])
                return t2

            thr_g = bcast_scaled(statg_res[0:1, 0:1], 2.0 ** -26, "thrg")
            m_g = bcast_scaled(statg_res[0:1, 0:1], 2.0 ** -25, "mg")
            nthr_g = colp.tile([128, 1], F32)
            nc.vector.tensor_scalar_mul(nthr_g[:], thr_g[:], -1.0)

            def cols_to_row_bcast(cols, name):
                with tc.tile_pool(name=f"psR{name}", bufs=1,
                                  space="PSUM") as psR:
                    ps = psR.tile([TT, 128], F32, tag="rowps",
                                  name=f"{name}_ps")
                    nc.tensor.transpose(ps[:], cols[:], ident_f[:])
                    r4 = colp.tile([TT, 128], F32, name=f"{name}_r4")
                    nc.scalar.copy(r4[:], ps[:])
                # bounce [4,128] -> [1,512] through DRAM (linear reinterp)
                rb = dram.tile([TT, 128], F32, name=f"{name}_rb")
                nc.gpsimd.dma_start(rb[:], r4[:])
                row = colp.tile([1, TPC], F32, name=f"{name}_r")
                nc.gpsimd.dma_start(
                    row[:],
                    rb[:].rearrange("a b -> (a b)").rearrange(
                        "(o f) -> o f", o=1))
                bc = colp.tile([128, TPC], F32, name=f"{name}_bc")
                nc.gpsimd.partition_broadcast(bc[:], row[:])
                return bc

            # mm1 per-token scale: emitted HERE so its Pool-queue bounce
            # runs before the tern writes, well ahead of the first drain.
            ys_cols = colp.tile([128, TT], F32)
            nc.vector.tensor_scalar(ys_cols[:], amax1[:], m_g[:], 1.0 / 127.0,
                                    Alu.mult, Alu.mult)
            ys_bc = cols_to_row_bcast(ys_cols, "ys")

            # ---- down-proj stats (streamed on the scalar ring) ----
            parts_d = colp.tile([128, DSH // 128], F32)
            for i in range(DSH // 128):
                wch = stp.tile([128, H], F32, tag="wst", name=f"wd{i}")
                nc.scalar.dma_start(wch[:], wd_ap[i * 128:(i + 1) * 128, :])
                nc.vector.tensor_reduce(parts_d[:, i:i + 1], wch[:], axis=X,
                                        op=Alu.add, apply_absolute_value=True)
            sums_d = colp.tile([128, 1], F32)
            nc.vector.tensor_reduce(sums_d[:], parts_d[:], axis=X, op=Alu.add)
            psd = psS.tile([1, 1], F32, tag="psd")
            nc.tensor.matmul(psd[:], sums_d[:], ones[:], start=True, stop=True)
            nc.scalar.copy(stat_d[:], psd[:])
            sd_in = dram.tile([1, 1], F32, name="sdin")
            sd_out = dram.tile([8, 1], F32, addr_space="Shared", name="sdout")
            nc.sync.dma_start(sd_in[:], stat_d[:])
            psS_ctx.__exit__(None, None, None)
            stp_ctx.__exit__(None, None, None)

            # ---- P2: ternarize gate (fp8), column-sharded. Chunks 0-1
            # on DVE, 2-5 on Pool; fp8 writes ride the SWDGE ring; each
            # chunk's AG trigger is emitted (Pool queue) right after the
            # chunk's last write so the cc stream runs in chunk order.
            tg_shs = [dram.tile([GSH, 2 * CW[j]], F8, name=f"tgsh{j}")
                      for j in range(NCH)]
            tg_fulls = [dram.tile([NC_N * GSH, 2 * CW[j]], F8,
                                  addr_space="Shared", name=f"tgf{j}")
                        for j in range(NCH)]
            td_sh = dram.tile([DSH, H], F8)
            td_full = dram.tile([I, H], F8, addr_space="Shared")

            tp_ctx = tc.tile_pool(name="ternp", bufs=2)
            tp = tp_ctx.__enter__()
            tb_ctx = tc.tile_pool(name="ternb", bufs=1)
            tb_p = tb_ctx.__enter__()

            def tern_flat(w_src, ncol, thr, nthr, nm):
                # elementwise ternarize a flat [128, ncol] f32 SBUF view
                # (DVE; big tiles -> 3 ops per call, not per row block);
                # returns the f8 tile (caller DMAs it out)
                a = tb_p.tile([128, ncol], BF16, tag="ta", name=f"ta{nm}")
                nc.vector.tensor_scalar(a[:], w_src, thr[:], 0.5,
                                        Alu.is_gt, Alu.subtract)
                b = tb_p.tile([128, ncol], BF16, tag="tb", name=f"tb{nm}")
                nc.vector.tensor_scalar(b[:], w_src, nthr[:], 0.5,
                                        Alu.is_ge, Alu.subtract)
                t = tb_p.tile([128, ncol], F8, tag="tc", name=f"tc{nm}")
                nc.vector.tensor_tensor(t[:], a[:], b[:], Alu.add)
                return t

            TFL = 4096  # flat tern tile free size

            def tern_gate_chunk(j, rd_eng):
                w = CW[j]
                npc = (HK * 2 * w) // TFL  # pieces per chunk (1 or 2)
                ipp = HK // npc            # i-blocks per piece
                wgv_g = wg_ap[:, GO[j]:GO[j] + w].rearrange(
                    "(i p) o -> p i o", p=128)
                wgv_v = wg_ap[:, GCW + GO[j]:GCW + GO[j] + w].rearrange(
                    "(i p) o -> p i o", p=128)
                for pc in range(npc):
                    i0 = pc * ipp
                    wch = tp.tile([128, ipp, 2 * w], F32, tag="tw",
                                  name=f"tw{j}_{pc}")
                    rd_eng.dma_start(wch[:, :, 0:w], wgv_g[:, i0:i0 + ipp])
                    rd_eng.dma_start(wch[:, :, w:2 * w],
                                     wgv_v[:, i0:i0 + ipp])
                    t = tern_flat(
                        wch[:].rearrange("p i o -> p (i o)"), ipp * 2 * w,
                        thr_g, nthr_g, f"g{j}_{pc}")
                    # plain-slice dst DMAs: a rearranged-view write is
                    # not reliably linked to the AG's input dependency
                    for ii in range(ipp):
                        i = i0 + ii
                        nc.gpsimd.dma_start(
                            tg_shs[j][i * 128:(i + 1) * 128, :],
                            t[:, ii * 2 * w:(ii + 1) * 2 * w])
                nc.gpsimd.collective_compute(
                    "AllGather", Alu.bypass, replica_groups=rg,
                    ins=[tg_shs[j][:]], outs=[tg_fulls[j][:]])

            tern_gate_chunk(0, nc.sync)
            tern_gate_chunk(1, nc.sync)

            # down stats AG trigger here: cc slot right after gate AG1,
            # so thr_d lands well before the mid-mm1 down tern.
            nc.gpsimd.collective_compute("AllGather", Alu.bypass,
                                         replica_groups=rg,
                                         ins=[sd_in[:]], outs=[sd_out[:]])

            tern_gate_chunk(2, nc.sync)
            tern_gate_chunk(3, nc.sync)
            tern_gate_chunk(4, nc.scalar)
            tern_gate_chunk(5, nc.scalar)

            # ---- down thresholds + down tern: emitted INSIDE the mm1
            # loop (at chunk 3) so the scalar-ring twd reads never dam
            # the h-spill pipeline behind a thr_d-gated WAR. The [8,1]
            # AG result is summed as a row reinterp (no PSUM needed:
            # psMM1 owns all 8 banks by then). ----
            m_d_box = []

            def emit_down_path():
                resd_row = colp.tile([1, 8], F32, name="resdrow")
                nc.gpsimd.dma_start(
                    resd_row[:],
                    sd_out[:].rearrange("a b -> (a b)").rearrange(
                        "(o f) -> o f", o=1))
                statd_res = colp.tile([1, 1], F32, name="statdres")
                nc.vector.tensor_reduce(statd_res[:], resd_row[:], axis=X,
                                        op=Alu.add)
                thr_d = bcast_scaled(statd_res[0:1, 0:1], 2.0 ** -25, "thrd")
                m_d_box.append(bcast_scaled(statd_res[0:1, 0:1], 2.0 ** -24,
                                            "md"))
                nthr_d = colp.tile([128, 1], F32)
                nc.vector.tensor_scalar_mul(nthr_d[:], thr_d[:], -1.0)
                wdv = wd_ap[:].rearrange("(i p) o -> p i o", p=128)
                for pc in range(4):
                    i0 = pc * 2
                    wch = tp.tile([128, 2, H], F32, tag="tw",
                                  name=f"twd{pc}")
                    nc.scalar.dma_start(wch[:], wdv[:, i0:i0 + 2])
                    t = tern_flat(
                        wch[:].rearrange("p i o -> p (i o)"), 2 * H,
                        thr_d, nthr_d, f"d{pc}")
                    for ii in range(2):
                        i = i0 + ii
                        nc.gpsimd.dma_start(
                            td_sh[i * 128:(i + 1) * 128, :],
                            t[:, ii * H:(ii + 1) * H])
                nc.gpsimd.collective_compute(
                    "AllGather", Alu.bypass, replica_groups=rg,
                    ins=[td_sh[:]], outs=[td_full[:]])

            # ---- P3: mm1 + SwiGLU -> h [I, tok] f32 spilled to DRAM --
            h_dram = dram.tile([I, TPC], F32)
            acc_sq = colp.tile([128, TPC], F32)
            nc.vector.memset(acc_sq[:], 0.0)
            acc_mxp = colp.tile([128, TPC], F32)
            nc.vector.memset(acc_mxp[:], -3.0e38)
            acc_mxn = colp.tile([128, TPC], F32)
            nc.vector.memset(acc_mxn[:], 3.0e38)
            tgvs = [t[:].rearrange("(c i p) o -> p c i o", p=128, i=HK)
                    for t in tg_fulls]

            with tc.tile_pool(name="p3", bufs=3) as p3, \
                 tc.tile_pool(name="hjp", bufs=6) as hjp, \
                 tc.tile_pool(name="psMM1", bufs=2, space="PSUM") as psM1:
              for j in range(NCH):
                w = CW[j]
                nb = w // 128
                for c in range(NC_N):
                    if j == 3 and c == 0:
                        emit_down_path()
                    tg_t = p3.tile([128, HK, 2 * w], F8, tag=f"tg{w}",
                                   name=f"tg{j}_{c}")
                    nc.sync.dma_start(tg_t[:], tgvs[j][:, c])
                    pg = [psM1.tile([128, TPC], F32, tag=f"pg{bb}",
                                    name=f"pg{j}_{c}_{bb}")
                          for bb in range(nb)]
                    pv = [psM1.tile([128, TPC], F32, tag=f"pv{bb}",
                                    name=f"pv{j}_{c}_{bb}")
                          for bb in range(nb)]
                    for i in range(HK):
                        rhs = xqT[:, i * TPC:(i + 1) * TPC]
                        st, sp = i == 0, i == HK - 1
                        for bb in range(nb):
                            co = bb * 128
                            nc.tensor.matmul(
                                pg[bb][:], tg_t[:, i, co:co + 128],
                                rhs, start=st, stop=sp)
                            nc.tensor.matmul(
                                pv[bb][:], tg_t[:, i, w + co:w + co + 128],
                                rhs, start=st, stop=sp)
                    for bb in range(nb):
                        jb = c * 8 + GO[j] // 128 + bb
                        gsc = p3.tile([128, TPC], F32, tag="gsc",
                                      name=f"gs{jb}")
                        nc.vector.tensor_tensor(gsc[:], pg[bb][:], ys_bc[:],
                                                Alu.mult)
                        sg = p3.tile([128, TPC], F32, tag="sg", name=f"sg{jb}")
                        nc.scalar.activation(sg[:], gsc[:], Act.Silu)
                        vsc = p3.tile([128, TPC], F32, tag="vsc",
                                      name=f"vs{jb}")
                        nc.vector.tensor_tensor(vsc[:], pv[bb][:], ys_bc[:],
                                                Alu.mult)
                        hj = hjp.tile([128, TPC], F32, tag="hj",
                                      name=f"hj{jb}")
                        nc.vector.tensor_tensor(hj[:], sg[:], vsc[:], Alu.mult)
                        seng = nc.scalar if (c + bb) % 2 == 0 else nc.sync
                        seng.dma_start(h_dram[jb * 128:(jb + 1) * 128, :],
                                       hj[:])
                        hsq = p3.tile([128, TPC], F32, tag="hsq",
                                      name=f"hq{jb}")
                        nc.scalar.activation(hsq[:], hj[:], Act.Square)
                        nc.vector.tensor_tensor(acc_sq[:], acc_sq[:], hsq[:],
                                                Alu.add)
                        hg = p3.tile([128, TPC], F32, tag="hg", name=f"hg{jb}")
                        nc.vector.tensor_scalar_mul(hg[:], hj[:],
                                                    gdc_sb[:, jb:jb + 1])
                        nc.vector.tensor_tensor(acc_mxp[:], acc_mxp[:], hg[:],
                                                Alu.max)
                        nc.vector.tensor_tensor(acc_mxn[:], acc_mxn[:], hg[:],
                                                Alu.min)

            tb_ctx.__exit__(None, None, None)
            tp_ctx.__exit__(None, None, None)
            xq_p.__exit__(None, None, None)

            # ---- P4: per-token stats over I (col form) ----
            # combine +/- max accumulators first: amax|hg| = max(mxp,-mxn)
            am_comb = colp.tile([128, TPC], F32)
            nc.vector.tensor_scalar_mul(am_comb[:], acc_mxn[:], -1.0)
            nc.vector.tensor_tensor(am_comb[:], am_comb[:], acc_mxp[:],
                                    Alu.max)
            qs_cols = colp.tile([128, TT], F32)
            y2s_cols = colp.tile([128, TT], F32)
            ps4_ctx = tc.tile_pool(name="ps4", bufs=2, space="PSUM")
            ps4 = ps4_ctx.__enter__()
            for t in range(TT):
                sl = slice(t * 128, (t + 1) * 128)
                pssq = ps4.tile([128, 128], F32, tag="pssq", name=f"pq{t}")
                nc.tensor.transpose(pssq[:], acc_sq[:, sl], ident_f[:])
                ss = colp.tile([128, 1], F32, name=f"hss{t}")
                nc.vector.tensor_reduce(ss[:], pssq[:], axis=X, op=Alu.add)
                psm = ps4.tile([128, 128], F32, tag="psm", name=f"pm{t}")
                nc.tensor.transpose(psm[:], am_comb[:, sl], ident_f[:])
                amax_hg = colp.tile([128, 1], F32, name=f"amhg{t}")
                nc.vector.tensor_reduce(amax_hg[:], psm[:], axis=X,
                                        op=Alu.max)
                sd2 = colp.tile([128, 1], F32, name=f"sd2{t}")
                nc.scalar.activation(sd2[:], ss[:], Act.Sqrt, bias=epsb[:],
                                     scale=1.0 / I)
                rstd2 = colp.tile([128, 1], F32, name=f"rstd2{t}")
                nc.vector.reciprocal(rstd2[:], sd2[:])
                t1 = colp.tile([128, 1], F32, name=f"t1{t}")
                nc.vector.tensor_scalar(t1[:], amax_hg[:], rstd2[:], EPS,
                                        Alu.mult, Alu.max)
                rc2 = colp.tile([128, 1], F32, name=f"rc2{t}")
                nc.vector.reciprocal(rc2[:], t1[:])
                s2 = colp.tile([128, 1], F32, name=f"s2{t}")
                nc.vector.tensor_scalar_mul(s2[:], rc2[:], 127.0)
                nc.vector.tensor_scalar_mul(qs_cols[:, t:t + 1], rstd2[:],
                                            s2[:])
                nc.vector.tensor_scalar(y2s_cols[:, t:t + 1], t1[:],
                                        m_d_box[0][:], 1.0 / 127.0,
                                        Alu.mult, Alu.mult)
            ps4_ctx.__exit__(None, None, None)
            qs_bc = cols_to_row_bcast(qs_cols, "qs")

            # ---- P5: j-outer mm2, 2 H-halves; quant fused in half 0,
            # q2T cached in SBUF (bf16) for half 1 ----
            tdv = td_full[:].rearrange("(j p) o -> p j o", p=128)
            q2p_ctx = tc.tile_pool(name="q2p", bufs=1)
            q2p = q2p_ctx.__enter__()
            q2T = q2p.tile([128, IK * TPC], BF16)
            with tc.tile_pool(name="p5", bufs=3) as p5, \
                 tc.tile_pool(name="psMM2", bufs=1, space="PSUM") as psM2:
                for half in range(2):
                    p2 = [psM2.tile([128, 512], F32, tag=f"p2_{q}",
                                    name=f"p2_{half}_{q}") for q in range(8)]
                    for j in range(IK):
                        if half == 0:
                            hjl = p5.tile([128, TPC], F32, tag="hjl",
                                          name=f"h4_{j}")
                            nc.sync.dma_start(
                                hjl[:], h_dram[j * 128:(j + 1) * 128, :])
                            hg2 = p5.tile([128, TPC], F32, tag="hg2",
                                          name=f"g4_{j}")
                            nc.scalar.activation(hg2[:], hjl[:], Act.Copy,
                                                 scale=gdc_sb[:, j:j + 1])
                            hq2 = p5.tile([128, TPC], F32, tag="hq2",
                                          name=f"q4_{j}")
                            nc.vector.tensor_tensor(hq2[:], hg2[:], qs_bc[:],
                                                    Alu.mult)
                            nc.vector.tensor_scalar(
                                q2T[:, j * TPC:(j + 1) * TPC], hq2[:],
                                C_MAGIC, C_MAGIC, Alu.add, Alu.subtract)
                        td_j = p5.tile([128, 1024], F8, tag="td",
                                       name=f"td{half}_{j}")
                        nc.scalar.dma_start(
                            td_j[:], tdv[:, j, half * 1024:(half + 1) * 1024])
                        st, sp = j == 0, j == IK - 1
                        for t in range(TT):
                            for hq in range(2):
                                nc.tensor.matmul(
                                    p2[t * 2 + hq][:],
                                    q2T[:, j * TPC + t * 128:
                                        j * TPC + (t + 1) * 128],
                                    td_j[:, hq * 512:(hq + 1) * 512],
                                    start=st, stop=sp)
                    for t in range(TT):
                        for hq in range(2):
                            yt = p5.tile([128, 512], F32, tag="yt",
                                         name=f"yt{half}_{t}_{hq}")
                            if hq == 0:
                                nc.vector.tensor_scalar_mul(
                                    yt[:], p2[t * 2 + hq][:],
                                    y2s_cols[:, t:t + 1])
                            else:
                                nc.scalar.activation(
                                    yt[:], p2[t * 2 + hq][:], Act.Copy,
                                    scale=y2s_cols[:, t:t + 1])
                            oeng = nc.sync if hq == 0 else nc.scalar
                            oeng.dma_start(
                                y_ap[t * 128:(t + 1) * 128,
                                     half * 1024 + hq * 512:
                                     half * 1024 + (hq + 1) * 512], yt[:])
            q2p_ctx.__exit__(None, None, None)
    return nc


_CACHE = {}


def _get_compiled():
    if "nc" not in _CACHE:
        nc = bacc.Bacc("TRN2", target_bir_lowering=False, debug=False,
                       enable_asserts=False, num_devices=NC_N)
        build(nc)
        nc.compile()
        _CACHE["nc"] = nc
    return _CACHE["nc"]


def make_in_maps(x, w_gate, g_gate, w_down, g_down):
    x2 = np.ascontiguousarray(np.asarray(x, np.float32).reshape(TOK, H))
    wgT = np.asarray(w_gate, np.float32).T          # [H, 2I]
    wdT = np.asarray(w_down, np.float32).T          # [I, H]
    gg = np.ascontiguousarray(np.asarray(g_gate, np.float32).reshape(1, H))
    gdc = np.ascontiguousarray(
        np.asarray(g_down, np.float32).reshape(IK, 128).T)
    in_maps = []
    for c in range(NC_N):
        wg_slice = np.concatenate(
            [wgT[:, c * GCW:(c + 1) * GCW],
             wgT[:, I + c * GCW:I + (c + 1) * GCW]], axis=1)
        in_maps.append({
            "x": x2[c * TPC:(c + 1) * TPC],
            "wgt": np.ascontiguousarray(wg_slice),
            "wdt": np.ascontiguousarray(wdT[c * DSH:(c + 1) * DSH]),
            "gg": gg,
            "gdc": gdc,
        })
    return in_maps


def kernel(x, w_gate, g_gate, w_down, g_down):
    nc = _get_compiled()
    in_maps = make_in_maps(x, w_gate, g_gate, w_down, g_down)
    res = run_bass_kernel_spmd(nc, in_maps, core_ids=list(range(NC_N)))
    out = np.concatenate([res.results[c]["y"] for c in range(NC_N)], axis=0)
    return out.reshape(B, S, H).astype(np.float32)


# revision 42
# speedup vs baseline: 1.0103x; 1.0103x over previous
"""HGRNBitMLP (BitNet-style SwiGLU MLP) on 8 TRN2 NeuronCores.

Data-parallel over the 4096 tokens (512/core). Weight ternarization is
COLUMN-sharded: core c ternarizes gate cols [c*1024,(c+1)*1024) plus the
matching v cols, so the per-chunk AllGathers fire as soon as the local
tern finishes (~80us) instead of waiting for a row-wise union. Ternary
weights ride fp8e4 (exact for {-1,0,+1}); the global mean(|w|) comes
from two tiny AllGathers (gate stats first so the gate tern never waits
on the down-proj read). Activations are quantized to the int8 grid
(exact in bf16), so every matmul is an exact-integer fp8xbf16 matmul
with f32 PSUM accumulation; per-token scales are applied outside.

Engine-queue discipline (strict FIFO per engine):
 - sync(SP) ring:   w-gate stat rows i<8 (held in SBUF for tern) ->
                    gate tern re-reads rows i>=8 chunks 0-3 ->
                    mm1 stationary tiles -> mm2 h reads + y evens.
 - scalar(ACT) ring: x -> w-gate stat rows i>=8 -> w-down (held) ->
                    gate tern re-reads chunks 4-5 -> mm1 h spills ->
                    mm2 td loads + y odds.
 - Pool queue: collective triggers (in chunk order), tern chunks 2-5,
                    down stats ... down tern, fp8 tern writes (SWDGE),
                    mm2 magic-rounds, broadcasts.
 - DVE: P0x helpers -> tern chunks 0-1 -> down-stat reduces -> mm1
                    SwiGLU drains -> P4 -> mm2 quant muls + yt scales.

Layouts: x is loaded [tok, h], quantized, PE-transposed to xqT [h, tok].
mm1 produces y^T tiles [o, tok]; SwiGLU keeps h as [I, tok] (f32,
spilled to DRAM) so mm2's operand q2T [I, tok] needs no transpose.
mm2 is j-outer over I with quantization fused into the first H-half
(q2T cached in SBUF bf16 for the second half), out [tok, H] in halves.
"""
import sys

try:
    import concourse  # noqa: F401
except ImportError:
    sys.path.insert(0, "/opt/trn_rl_repo")

import numpy as np

import concourse.tile as tile
from concourse import bacc, mybir
from concourse.bass_utils import run_bass_kernel_spmd
from concourse.masks import make_identity

F32, BF16, F8 = mybir.dt.float32, mybir.dt.bfloat16, mybir.dt.float8e4
Alu = mybir.AluOpType
Act = mybir.ActivationFunctionType
X = mybir.AxisListType.X

NC_N = 8
B, S, H, I = 2, 2048, 2048, 8192
O2 = 2 * I
TOK = B * S
TPC = TOK // NC_N   # 512 tokens/core
TT = TPC // 128     # 4 token tiles
HK = H // 128       # 16 h tiles
IK = I // 128       # 64 I tiles
GSH = H             # rows of the (column-sharded) gate slice
GCW = I // NC_N     # 1024 gate cols (and v cols) per core
DSH = I // NC_N     # 1024 rows of w_down^T per core
EPS = 1e-5
C_MAGIC = 12582912.0  # 1.5*2^23; (x+C)-C rounds f32 to nearest-even int

# gate AG chunks: gate-col widths (same width of v cols rides along)
CW = [128, 128, 128, 128, 256, 256]
GO = [0, 128, 256, 384, 512, 768]
NCH = len(CW)


def build(nc):
    x_ap = nc.dram_tensor("x", [TPC, H], F32, kind="ExternalInput").ap()
    wg_ap = nc.dram_tensor("wgt", [GSH, 2 * GCW], F32, kind="ExternalInput").ap()
    wd_ap = nc.dram_tensor("wdt", [DSH, H], F32, kind="ExternalInput").ap()
    gg_ap = nc.dram_tensor("gg", [1, H], F32, kind="ExternalInput").ap()
    gd_ap = nc.dram_tensor("gdc", [128, IK], F32, kind="ExternalInput").ap()
    y_ap = nc.dram_tensor("y", [TPC, H], F32, kind="ExternalOutput").ap()
    rg = [list(range(NC_N))]

    with tile.TileContext(nc) as tc:
        with tc.tile_pool(name="dram", bufs=1, space="DRAM") as dram, \
             tc.tile_pool(name="perm", bufs=1) as cp, \
             tc.tile_pool(name="colp", bufs=1) as colp:

            # ---- inits (before any collective hits the Pool queue) --
            ones = cp.tile([128, 1], F32)
            nc.gpsimd.memset(ones[:], 1.0)
            epsb = cp.tile([128, 1], F32)
            nc.gpsimd.memset(epsb[:], EPS)
            ident_b = cp.tile([128, 128], BF16)
            make_identity(nc, ident_b[:])
            ident_f = cp.tile([128, 128], F32)
            make_identity(nc, ident_f[:])
            gdc_sb = cp.tile([128, IK], F32)
            nc.gpsimd.dma_start(gdc_sb[:], gd_ap[:])
            gg_sb = cp.tile([1, H], F32)
            nc.gpsimd.dma_start(gg_sb[:], gg_ap[:])
            ones_row = cp.tile([1, 128], F32)
            nc.gpsimd.memset(ones_row[:], 1.0)

            # partition-broadcast via PE ones-matmul: the gpsimd DSP
            # broadcast runs at ~10 M elem/s and would stall the chain
            def pe_broadcast(dst, row_ap, n, psp, nm):
                for ck in range((n + 511) // 512):
                    c0, c1 = ck * 512, min((ck + 1) * 512, n)
                    ps = psp.tile([128, 512], F32, tag="bcps",
                                  name=f"bc{nm}{ck}")
                    nc.tensor.matmul(ps[:, 0:c1 - c0], ones_row[:],
                                     row_ap[0:1, c0:c1], start=True,
                                     stop=True)
                    nc.scalar.copy(dst[:, c0:c1], ps[:, 0:c1 - c0])

            g_bc = cp.tile([128, H], F32)
            with tc.tile_pool(name="psBg", bufs=2, space="PSUM") as psBg:
                pe_broadcast(g_bc, gg_sb[:], H, psBg, "g")

            # warmup collective: the first collective after the init
            # barrier pays ~20us extra; burn that on a dummy.
            warm_sb = cp.tile([1, 1], F32)
            nc.vector.memset(warm_sb[:], 0.0)
            warm_in = dram.tile([1, 1], F32, name="warm_in")
            warm_out = dram.tile([8, 1], F32, addr_space="Shared",
                                 name="warm_out")
            nc.sync.dma_start(warm_in[:], warm_sb[:])
            nc.gpsimd.collective_compute("AllGather", Alu.bypass,
                                         replica_groups=rg,
                                         ins=[warm_in[:]], outs=[warm_out[:]])

            # ---- P0x: x rmsnorm + int8-grid quant + transpose.
            # Emitted FIRST on ACT/DVE so the x chain races ahead of the
            # stats reduces; x rides the scalar ring before the w tiles.
            xq_p = tc.tile_pool(name="xqp", bufs=1)
            xq_pool = xq_p.__enter__()
            xqT = xq_pool.tile([128, HK * TPC], BF16)
            amax1 = colp.tile([128, TT], F32)
            # statp opens BEFORE xwork so its tiles get their own SBUF
            # region: reusing xwork's space would gate every stats DMA
            # on P0x completion (WAR) and stall the whole startup.
            stp_ctx = tc.tile_pool(name="statp", bufs=6)
            stp = stp_ctx.__enter__()
            xw_ctx = tc.tile_pool(name="xwork", bufs=2)
            xw = xw_ctx.__enter__()
            psX_ctx = tc.tile_pool(name="psX", bufs=2, space="PSUM")
            psX = psX_ctx.__enter__()
            for t in range(TT):
                xt = xw.tile([128, H], F32, tag="xt", name=f"xt{t}")
                nc.scalar.dma_start(xt[:], x_ap[t * 128:(t + 1) * 128, :])
                ssq = colp.tile([128, 1], F32, name=f"ssq{t}")
                scr = xw.tile([128, H], F32, tag="scrA", name=f"sq{t}")
                nc.scalar.activation(scr[:], xt[:], Act.Square,
                                     accum_out=ssq[:])
                sd = colp.tile([128, 1], F32, name=f"sd{t}")
                nc.scalar.activation(sd[:], ssq[:], Act.Sqrt, bias=epsb[:],
                                     scale=1.0 / H)
                rstd = colp.tile([128, 1], F32, name=f"rstd{t}")
                nc.vector.reciprocal(rstd[:], sd[:])
                # fold rstd into the final quant scale: xn' = x*g, then
                # q = round(xn' * (rstd*127/clip(amax|xn'|*rstd, eps)))
                xn = xw.tile([128, H], F32, tag="xn", name=f"xn{t}")
                nc.vector.tensor_tensor(xn[:], xt[:], g_bc[:], Alu.mult)
                am = amax1[:, t:t + 1]
                nc.vector.tensor_reduce(am, xn[:], axis=X, op=Alu.max,
                                        apply_absolute_value=True)
                # amax1 holds amax|xnorm| = amax|x*g| * rstd, clipped
                nc.vector.tensor_scalar(am, am, rstd[:], EPS,
                                        Alu.mult, Alu.max)
                rc = colp.tile([128, 1], F32, name=f"rc{t}")
                nc.vector.reciprocal(rc[:], am)
                s1 = colp.tile([128, 1], F32, name=f"s1{t}")
                nc.vector.tensor_scalar(s1[:], rc[:], rstd[:], 127.0,
                                        Alu.mult, Alu.mult)
                # int8-grid round via the magic-constant trick on ACT
                q1 = xw.tile([128, H], F32, tag="scrA", name=f"q1{t}")
                nc.scalar.activation(q1[:], xn[:], Act.Copy,
                                     scale=s1[:], bias=C_MAGIC)
                q = xw.tile([128, H], BF16, tag="q", name=f"q{t}")
                nc.scalar.activation(q[:], q1[:], Act.Copy, bias=-C_MAGIC)
                for i in range(HK):
                    tps = psX.tile([128, 128], BF16, tag="tps",
                                   name=f"tps{t}_{i}")
                    nc.tensor.transpose(tps[:], q[:, i * 128:(i + 1) * 128],
                                        ident_b[:])
                    dst = xqT[:, i * TPC + t * 128:i * TPC + (t + 1) * 128]
                    if i % 2 == 0:
                        nc.scalar.copy(dst, tps[:])
                    else:
                        nc.vector.tensor_copy(dst, tps[:])
            psX_ctx.__exit__(None, None, None)
            xw_ctx.__exit__(None, None, None)

            # ---- P0w: sharded abs-sums. Gate first (its own tiny AG)
            # so the gate tern never waits on the down-proj read. ----
            stat_g = colp.tile([1, 1], F32)
            stat_d = colp.tile([1, 1], F32)
            psS_ctx = tc.tile_pool(name="psStat", bufs=1, space="PSUM")
            psS = psS_ctx.__enter__()

            parts_g = colp.tile([128, HK], F32)
            for i in range(HK):
                wch = stp.tile([128, 2 * GCW], F32, tag="wst", name=f"ws{i}")
                # ring balance: sync is otherwise idle early; scalar
                # already carries x (down-proj reads come much later)
                eng = nc.sync if i < 10 else nc.scalar
                eng.dma_start(wch[:], wg_ap[i * 128:(i + 1) * 128, :])
                nc.vector.tensor_reduce(parts_g[:, i:i + 1], wch[:], axis=X,
                                        op=Alu.add, apply_absolute_value=True)
            sums_g = colp.tile([128, 1], F32)
            nc.vector.tensor_reduce(sums_g[:], parts_g[:], axis=X, op=Alu.add)
            psg = psS.tile([1, 1], F32, tag="psg")
            nc.tensor.matmul(psg[:], sums_g[:], ones[:], start=True, stop=True)
            nc.scalar.copy(stat_g[:], psg[:])
            sg_in = dram.tile([1, 1], F32, name="sgin")
            sg_out = dram.tile([8, 1], F32, addr_space="Shared", name="sgout")
            nc.sync.dma_start(sg_in[:], stat_g[:])
            nc.gpsimd.collective_compute("AllGather", Alu.bypass,
                                         replica_groups=rg,
                                         ins=[sg_in[:]], outs=[sg_out[:]])
            resg = colp.tile([8, 1], F32)
            nc.sync.dma_start(resg[:], sg_out[:])
            psg2 = psS.tile([1, 1], F32, tag="psg2")
            nc.tensor.matmul(psg2[:], ones[0:8, 0:1], resg[:],
                             start=True, stop=True)
            statg_res = colp.tile([1, 1], F32)
            nc.scalar.copy(statg_res[:], psg2[:])
            psS_ctx.__exit__(None, None, None)
            stp_ctx.__exit__(None, None, None)

            # ---- gate thresholds ----
            def bcast_scaled(src, scale, name):
                t1 = colp.tile([1, 1], F32, name=f"{name}_s")
                nc.vector.tensor_scalar_mul(t1[:], src, scale)
                t2 = colp.tile([128, 1], F32, name=f"{name}_b")
                nc.gpsimd.partition_broadcast(t2[:], t1[:])
                return t2

            thr_g = bcast_scaled(statg_res[0:1, 0:1], 2.0 ** -26, "thrg")
            m_g = bcast_scaled(statg_res[0:1, 0:1], 2.0 ** -25, "mg")
            nthr_g = colp.tile([128, 1], F32)
            nc.vector.tensor_scalar_mul(nthr_g[:], thr_g[:], -1.0)

            def cols_to_row_bcast(cols, name):
                with tc.tile_pool(name=f"psR{name}", bufs=1,
                                  space="PSUM") as psR:
                    ps = psR.tile([TT, 128], F32, tag="rowps",
                                  name=f"{name}_ps")
                    nc.tensor.transpose(ps[:], cols[:], ident_f[:])
                    r4 = colp.tile([TT, 128], F32, name=f"{name}_r4")
                    nc.scalar.copy(r4[:], ps[:])
                # bounce [4,128] -> [1,512] through DRAM (linear reinterp)
                rb = dram.tile([TT, 128], F32, name=f"{name}_rb")
                nc.gpsimd.dma_start(rb[:], r4[:])
                row = colp.tile([1, TPC], F32, name=f"{name}_r")
                nc.gpsimd.dma_start(
                    row[:],
                    rb[:].rearrange("a b -> (a b)").rearrange(
                        "(o f) -> o f", o=1))
                bc = colp.tile([128, TPC], F32, name=f"{name}_bc")
                with tc.tile_pool(name=f"psB{name}", bufs=1,
                                  space="PSUM") as psB:
                    pe_broadcast(bc, row[:], TPC, psB, name)
                return bc

            # mm1 per-token scale: emitted HERE so its Pool-queue bounce
            # runs before the tern writes, well ahead of the first drain.
            ys_cols = colp.tile([128, TT], F32)
            nc.vector.tensor_scalar(ys_cols[:], amax1[:], m_g[:], 1.0 / 127.0,
                                    Alu.mult, Alu.mult)
            ys_bc = cols_to_row_bcast(ys_cols, "ys")
            sd_in = dram.tile([1, 1], F32, name="sdin")
            sd_out = dram.tile([8, 1], F32, addr_space="Shared", name="sdout")

            # ---- P2: ternarize gate (fp8), column-sharded. Chunks 0-1
            # on DVE, 2-5 on Pool; fp8 writes ride the SWDGE ring; each
            # chunk's AG trigger is emitted (Pool queue) right after the
            # chunk's last write so the cc stream runs in chunk order.
            tg_shs = [dram.tile([GSH, 2 * CW[j]], F8, name=f"tgsh{j}")
                      for j in range(NCH)]
            tg_fulls = [dram.tile([NC_N * GSH, 2 * CW[j]], F8,
                                  addr_space="Shared", name=f"tgf{j}")
                        for j in range(NCH)]
            td_sh = dram.tile([DSH, H], F8)
            td_full = dram.tile([I, H], F8, addr_space="Shared")

            tp_ctx = tc.tile_pool(name="ternp", bufs=2)
            tp = tp_ctx.__enter__()
            tb_ctx = tc.tile_pool(name="ternb", bufs=1)
            tb_p = tb_ctx.__enter__()

            def tern_flat(w_src, ncol, thr, nthr, nm):
                # elementwise ternarize a flat [128, ncol] f32 SBUF view
                # (DVE; big tiles -> 3 ops per call, not per row block);
                # returns the f8 tile (caller DMAs it out)
                a = tb_p.tile([128, ncol], BF16, tag="ta", name=f"ta{nm}")
                nc.vector.tensor_scalar(a[:], w_src, thr[:], 0.5,
                                        Alu.is_gt, Alu.subtract)
                b = tb_p.tile([128, ncol], BF16, tag="tb", name=f"tb{nm}")
                nc.vector.tensor_scalar(b[:], w_src, nthr[:], 0.5,
                                        Alu.is_ge, Alu.subtract)
                t = tb_p.tile([128, ncol], F8, tag="tc", name=f"tc{nm}")
                nc.vector.tensor_tensor(t[:], a[:], b[:], Alu.add)
                return t

            TFL = 4096  # flat tern tile free size

            def tern_gate_chunk(j, rd_eng):
                w = CW[j]
                npc = (HK * 2 * w) // TFL  # pieces per chunk (1 or 2)
                ipp = HK // npc            # i-blocks per piece
                wgv_g = wg_ap[:, GO[j]:GO[j] + w].rearrange(
                    "(i p) o -> p i o", p=128)
                wgv_v = wg_ap[:, GCW + GO[j]:GCW + GO[j] + w].rearrange(
                    "(i p) o -> p i o", p=128)
                for pc in range(npc):
                    i0 = pc * ipp
                    wch = tp.tile([128, ipp, 2 * w], F32, tag="tw",
                                  name=f"tw{j}_{pc}")
                    rd_eng.dma_start(wch[:, :, 0:w], wgv_g[:, i0:i0 + ipp])
                    rd_eng.dma_start(wch[:, :, w:2 * w],
                                     wgv_v[:, i0:i0 + ipp])
                    t = tern_flat(
                        wch[:].rearrange("p i o -> p (i o)"), ipp * 2 * w,
                        thr_g, nthr_g, f"g{j}_{pc}")
                    # plain-slice dst DMAs: a rearranged-view write is
                    # not reliably linked to the AG's input dependency
                    for ii in range(ipp):
                        i = i0 + ii
                        nc.gpsimd.dma_start(
                            tg_shs[j][i * 128:(i + 1) * 128, :],
                            t[:, ii * 2 * w:(ii + 1) * 2 * w])
                nc.gpsimd.collective_compute(
                    "AllGather", Alu.bypass, replica_groups=rg,
                    ins=[tg_shs[j][:]], outs=[tg_fulls[j][:]])

            tern_gate_chunk(0, nc.sync)
            tern_gate_chunk(1, nc.sync)
            tern_gate_chunk(2, nc.sync)
            tern_gate_chunk(3, nc.sync)
            tern_gate_chunk(4, nc.scalar)
            tern_gate_chunk(5, nc.scalar)

            # ---- down-proj stats: read in flat [128, 2, H] pairs on the
            # scalar ring (after the gate tern re-reads), reduced on DVE;
            # the partition sum rides a tiny DRAM row-reinterp bounce (no
            # PSUM -> the pool stack stays clean for mm1). The AG trigger
            # sits after the gate AG triggers so it never dams them. ----
            parts_d = colp.tile([128, 4], F32)
            wdv_s = wd_ap[:].rearrange("(i p) o -> p i o", p=128)
            for pc in range(4):
                wch = tp.tile([128, 2, H], F32, tag="tw", name=f"wds{pc}")
                nc.scalar.dma_start(wch[:], wdv_s[:, pc * 2:pc * 2 + 2])
                nc.vector.tensor_reduce(parts_d[:, pc:pc + 1],
                                        wch[:].rearrange("p i o -> p (i o)"),
                                        axis=X, op=Alu.add,
                                        apply_absolute_value=True)
            sums_d = colp.tile([128, 1], F32)
            nc.vector.tensor_reduce(sums_d[:], parts_d[:], axis=X, op=Alu.add)
            sdb = dram.tile([128, 1], F32, name="sdbounce")
            nc.gpsimd.dma_start(sdb[:], sums_d[:])
            sd_row = colp.tile([1, 128], F32, name="sdrow")
            nc.gpsimd.dma_start(
                sd_row[:],
                sdb[:].rearrange("a b -> (a b)").rearrange("(o f) -> o f",
                                                           o=1))
            nc.vector.tensor_reduce(stat_d[:], sd_row[:], axis=X, op=Alu.add)
            nc.sync.dma_start(sd_in[:], stat_d[:])
            nc.gpsimd.collective_compute("AllGather", Alu.bypass,
                                         replica_groups=rg,
                                         ins=[sd_in[:]], outs=[sd_out[:]])

            # ---- down thresholds + down tern: emitted INSIDE the mm1
            # loop (at chunk 3) so the scalar-ring twd reads never dam
            # the h-spill pipeline behind a thr_d-gated WAR. The [8,1]
            # AG result is summed as a row reinterp (no PSUM needed:
            # psMM1 owns all 8 banks by then). ----
            m_d_box = []

            def emit_down_path():
                resd_row = colp.tile([1, 8], F32, name="resdrow")
                nc.gpsimd.dma_start(
                    resd_row[:],
                    sd_out[:].rearrange("a b -> (a b)").rearrange(
                        "(o f) -> o f", o=1))
                statd_res = colp.tile([1, 1], F32, name="statdres")
                nc.vector.tensor_reduce(statd_res[:], resd_row[:], axis=X,
                                        op=Alu.add)
                thr_d = bcast_scaled(statd_res[0:1, 0:1], 2.0 ** -25, "thrd")
                m_d_box.append(bcast_scaled(statd_res[0:1, 0:1], 2.0 ** -24,
                                            "md"))
                nthr_d = colp.tile([128, 1], F32)
                nc.vector.tensor_scalar_mul(nthr_d[:], thr_d[:], -1.0)
                wdv = wd_ap[:].rearrange("(i p) o -> p i o", p=128)
                for pc in range(4):
                    i0 = pc * 2
                    wch = tp.tile([128, 2, H], F32, tag="tw",
                                  name=f"twd{pc}")
                    nc.sync.dma_start(wch[:], wdv[:, i0:i0 + 2])
                    t = tern_flat(
                        wch[:].rearrange("p i o -> p (i o)"), 2 * H,
                        thr_d, nthr_d, f"d{pc}")
                    for ii in range(2):
                        i = i0 + ii
                        nc.gpsimd.dma_start(
                            td_sh[i * 128:(i + 1) * 128, :],
                            t[:, ii * H:(ii + 1) * H])
                nc.gpsimd.collective_compute(
                    "AllGather", Alu.bypass, replica_groups=rg,
                    ins=[td_sh[:]], outs=[td_full[:]])

            # ---- P3: mm1 + SwiGLU -> h [I, tok] f32 spilled to DRAM --
            h_dram = dram.tile([I, TPC], F32)
            acc_sq = colp.tile([128, TPC], F32)
            nc.vector.memset(acc_sq[:], 0.0)
            acc_mxp = colp.tile([128, TPC], F32)
            nc.vector.memset(acc_mxp[:], -3.0e38)
            acc_mxn = colp.tile([128, TPC], F32)
            nc.vector.memset(acc_mxn[:], 3.0e38)
            tgvs = [t[:].rearrange("(c i p) o -> p c i o", p=128, i=HK)
                    for t in tg_fulls]

            with tc.tile_pool(name="p3", bufs=3) as p3, \
                 tc.tile_pool(name="hjp", bufs=6) as hjp, \
                 tc.tile_pool(name="psMM1", bufs=2, space="PSUM") as psM1:
              for j in range(NCH):
                w = CW[j]
                nb = w // 128
                for c in range(NC_N):
                    if j == 5 and c == 0:
                        emit_down_path()
                    tg_t = p3.tile([128, HK, 2 * w], F8, tag=f"tg{w}",
                                   name=f"tg{j}_{c}")
                    nc.sync.dma_start(tg_t[:], tgvs[j][:, c])
                    pg = [psM1.tile([128, TPC], F32, tag=f"pg{bb}",
                                    name=f"pg{j}_{c}_{bb}")
                          for bb in range(nb)]
                    pv = [psM1.tile([128, TPC], F32, tag=f"pv{bb}",
                                    name=f"pv{j}_{c}_{bb}")
                          for bb in range(nb)]
                    for i in range(HK):
                        rhs = xqT[:, i * TPC:(i + 1) * TPC]
                        st, sp = i == 0, i == HK - 1
                        for bb in range(nb):
                            co = bb * 128
                            nc.tensor.matmul(
                                pg[bb][:], tg_t[:, i, co:co + 128],
                                rhs, start=st, stop=sp)
                            nc.tensor.matmul(
                                pv[bb][:], tg_t[:, i, w + co:w + co + 128],
                                rhs, start=st, stop=sp)
                    for bb in range(nb):
                        jb = c * 8 + GO[j] // 128 + bb
                        gsc = p3.tile([128, TPC], F32, tag="gsc",
                                      name=f"gs{jb}")
                        nc.vector.tensor_tensor(gsc[:], pg[bb][:], ys_bc[:],
                                                Alu.mult)
                        sg = p3.tile([128, TPC], F32, tag="sg", name=f"sg{jb}")
                        nc.scalar.activation(sg[:], gsc[:], Act.Silu)
                        vsc = p3.tile([128, TPC], F32, tag="vsc",
                                      name=f"vs{jb}")
                        nc.vector.tensor_tensor(vsc[:], pv[bb][:], ys_bc[:],
                                                Alu.mult)
                        hj = hjp.tile([128, TPC], F32, tag="hj",
                                      name=f"hj{jb}")
                        nc.vector.tensor_tensor(hj[:], sg[:], vsc[:], Alu.mult)
                        seng = nc.scalar if (c + bb) % 2 == 0 else nc.sync
                        seng.dma_start(h_dram[jb * 128:(jb + 1) * 128, :],
                                       hj[:])
                        hsq = p3.tile([128, TPC], F32, tag="hsq",
                                      name=f"hq{jb}")
                        nc.scalar.activation(hsq[:], hj[:], Act.Square)
                        nc.vector.tensor_tensor(acc_sq[:], acc_sq[:], hsq[:],
                                                Alu.add)
                        hg = p3.tile([128, TPC], F32, tag="hg", name=f"hg{jb}")
                        nc.vector.tensor_scalar_mul(hg[:], hj[:],
                                                    gdc_sb[:, jb:jb + 1])
                        nc.vector.tensor_tensor(acc_mxp[:], acc_mxp[:], hg[:],
                                                Alu.max)
                        nc.vector.tensor_tensor(acc_mxn[:], acc_mxn[:], hg[:],
                                                Alu.min)

            tb_ctx.__exit__(None, None, None)
            tp_ctx.__exit__(None, None, None)
            xq_p.__exit__(None, None, None)

            # ---- P4: per-token stats over I (col form) ----
            # combine +/- max accumulators first: amax|hg| = max(mxp,-mxn)
            am_comb = colp.tile([128, TPC], F32)
            nc.vector.tensor_scalar_mul(am_comb[:], acc_mxn[:], -1.0)
            nc.vector.tensor_tensor(am_comb[:], am_comb[:], acc_mxp[:],
                                    Alu.max)
            qs_cols = colp.tile([128, TT], F32)
            y2s_cols = colp.tile([128, TT], F32)
            ps4_ctx = tc.tile_pool(name="ps4", bufs=2, space="PSUM")
            ps4 = ps4_ctx.__enter__()
            for t in range(TT):
                sl = slice(t * 128, (t + 1) * 128)
                pssq = ps4.tile([128, 128], F32, tag="pssq", name=f"pq{t}")
                nc.tensor.transpose(pssq[:], acc_sq[:, sl], ident_f[:])
                ss = colp.tile([128, 1], F32, name=f"hss{t}")
                nc.vector.tensor_reduce(ss[:], pssq[:], axis=X, op=Alu.add)
                psm = ps4.tile([128, 128], F32, tag="psm", name=f"pm{t}")
                nc.tensor.transpose(psm[:], am_comb[:, sl], ident_f[:])
                amax_hg = colp.tile([128, 1], F32, name=f"amhg{t}")
                nc.vector.tensor_reduce(amax_hg[:], psm[:], axis=X,
                                        op=Alu.max)
                sd2 = colp.tile([128, 1], F32, name=f"sd2{t}")
                nc.scalar.activation(sd2[:], ss[:], Act.Sqrt, bias=epsb[:],
                                     scale=1.0 / I)
                rstd2 = colp.tile([128, 1], F32, name=f"rstd2{t}")
                nc.vector.reciprocal(rstd2[:], sd2[:])
                t1 = colp.tile([128, 1], F32, name=f"t1{t}")
                nc.vector.tensor_scalar(t1[:], amax_hg[:], rstd2[:], EPS,
                                        Alu.mult, Alu.max)
                rc2 = colp.tile([128, 1], F32, name=f"rc2{t}")
                nc.vector.reciprocal(rc2[:], t1[:])
                s2 = colp.tile([128, 1], F32, name=f"s2{t}")
                nc.vector.tensor_scalar_mul(s2[:], rc2[:], 127.0)
                nc.vector.tensor_scalar_mul(qs_cols[:, t:t + 1], rstd2[:],
                                            s2[:])
                nc.vector.tensor_scalar(y2s_cols[:, t:t + 1], t1[:],
                                        m_d_box[0][:], 1.0 / 127.0,
                                        Alu.mult, Alu.mult)
            ps4_ctx.__exit__(None, None, None)
            qs_bc = cols_to_row_bcast(qs_cols, "qs")

            # ---- P5: j-outer mm2, 2 H-halves; quant fused in half 0,
            # q2T cached in SBUF (bf16) for half 1 ----
            tdv = td_full[:].rearrange("(j p) o -> p j o", p=128)
            q2p_ctx = tc.tile_pool(name="q2p", bufs=1)
            q2p = q2p_ctx.__enter__()
            q2T = q2p.tile([128, IK * TPC], BF16)
            with tc.tile_pool(name="p5", bufs=3) as p5, \
                 tc.tile_pool(name="psMM2", bufs=1, space="PSUM") as psM2:
                for half in range(2):
                    p2 = [psM2.tile([128, 512], F32, tag=f"p2_{q}",
                                    name=f"p2_{half}_{q}") for q in range(8)]
                    for j in range(IK):
                        if half == 0:
                            hjl = p5.tile([128, TPC], F32, tag="hjl",
                                          name=f"h4_{j}")
                            nc.sync.dma_start(
                                hjl[:], h_dram[j * 128:(j + 1) * 128, :])
                            hg2 = p5.tile([128, TPC], F32, tag="hg2",
                                          name=f"g4_{j}")
                            nc.scalar.activation(hg2[:], hjl[:], Act.Copy,
                                                 scale=gdc_sb[:, j:j + 1])
                            hq2 = p5.tile([128, TPC], F32, tag="hq2",
                                          name=f"q4_{j}")
                            nc.vector.tensor_tensor(hq2[:], hg2[:], qs_bc[:],
                                                    Alu.mult)
                            nc.vector.tensor_scalar(
                                q2T[:, j * TPC:(j + 1) * TPC], hq2[:],
                                C_MAGIC, C_MAGIC, Alu.add, Alu.subtract)
                        td_j = p5.tile([128, 1024], F8, tag="td",
                                       name=f"td{half}_{j}")
                        nc.scalar.dma_start(
                            td_j[:], tdv[:, j, half * 1024:(half + 1) * 1024])
                        st, sp = j == 0, j == IK - 1
                        for t in range(TT):
                            for hq in range(2):
                                nc.tensor.matmul(
                                    p2[t * 2 + hq][:],
                                    q2T[:, j * TPC + t * 128:
                                        j * TPC + (t + 1) * 128],
                                    td_j[:, hq * 512:(hq + 1) * 512],
                                    start=st, stop=sp)
                    for t in range(TT):
                        for hq in range(2):
                            yt = p5.tile([128, 512], F32, tag="yt",
                                         name=f"yt{half}_{t}_{hq}")
                            if hq == 0:
                                nc.vector.tensor_scalar_mul(
                                    yt[:], p2[t * 2 + hq][:],
                                    y2s_cols[:, t:t + 1])
                            else:
                                nc.scalar.activation(
                                    yt[:], p2[t * 2 + hq][:], Act.Copy,
                                    scale=y2s_cols[:, t:t + 1])
                            oeng = nc.sync if hq == 0 else nc.scalar
                            oeng.dma_start(
                                y_ap[t * 128:(t + 1) * 128,
                                     half * 1024 + hq * 512:
                                     half * 1024 + (hq + 1) * 512], yt[:])
            q2p_ctx.__exit__(None, None, None)
    return nc


_CACHE = {}


def _get_compiled():
    if "nc" not in _CACHE:
        nc = bacc.Bacc("TRN2", target_bir_lowering=False, debug=False,
                       enable_asserts=False, num_devices=NC_N)
        build(nc)
        nc.compile()
        _CACHE["nc"] = nc
    return _CACHE["nc"]


def make_in_maps(x, w_gate, g_gate, w_down, g_down):
    x2 = np.ascontiguousarray(np.asarray(x, np.float32).reshape(TOK, H))
    wgT = np.asarray(w_gate, np.float32).T          # [H, 2I]
    wdT = np.asarray(w_down, np.float32).T          # [I, H]
    gg = np.ascontiguousarray(np.asarray(g_gate, np.float32).reshape(1, H))
    gdc = np.ascontiguousarray(
        np.asarray(g_down, np.float32).reshape(IK, 128).T)
    in_maps = []
    for c in range(NC_N):
        wg_slice = np.concatenate(
            [wgT[:, c * GCW:(c + 1) * GCW],
             wgT[:, I + c * GCW:I + (c + 1) * GCW]], axis=1)
        in_maps.append({
            "x": x2[c * TPC:(c + 1) * TPC],
            "wgt": np.ascontiguousarray(wg_slice),
            "wdt": np.ascontiguousarray(wdT[c * DSH:(c + 1) * DSH]),
            "gg": gg,
            "gdc": gdc,
        })
    return in_maps


def kernel(x, w_gate, g_gate, w_down, g_down):
    nc = _get_compiled()
    in_maps = make_in_maps(x, w_gate, g_gate, w_down, g_down)
    res = run_bass_kernel_spmd(nc, in_maps, core_ids=list(range(NC_N)))
    out = np.concatenate([res.results[c]["y"] for c in range(NC_N)], axis=0)
    return out.reshape(B, S, H).astype(np.float32)
